# revision 20
# baseline (speedup 1.0000x reference)
"""Causal multi-head attention block (B=4, N=2048, C=768, H=12) on 8 TRN2 cores.

Sharding: 48 (batch, head) pairs -> core c handles batch c//2 and heads
[0..5] (even c) or [6..11] (odd c). Each core runs an identical Bass program
on host-pre-sliced inputs; partial projection outputs are summed pairwise on
the host (plus proj bias).

Per-core kernel (everything in transposed [feature, token] layout):
  - Q^T/K^T projection: lhsT = wqk^T chunks, rhs = x^T chunks (fp32r matmuls)
  - V projection in natural [token, feature] layout (lhsT = x^T chunk)
  - S^T = K Q^T per head with 2-head tile_position row packing (contraction 64)
  - causal masking via 4 precomputed additive mask variants
  - exp on ScalarE (scale=1/sqrt(HD) folded into the activation affine)
  - PV: out^T = V_aug^T P^T with V augmented by a ones column so the softmax
    denominator accumulates in psum row 64
  - normalize via one batched DVE reciprocal + DMA partition-broadcast
  - output projection to out^T [768, 2048] per core
"""

import os

import numpy as np

import concourse.bass as bass
import concourse.mybir as mybir
import concourse.tile as tile
from concourse import bacc
from concourse.bass_utils import run_bass_kernel_spmd

B, N, C = 4, 2048, 768
H, HD = 12, 64
HPC = 6  # heads per core
NCORES = 8
SCALE = HD ** -0.5
QC = 512  # q-chunk width
NQC = N // QC  # 4
NKB = N // 128  # 16 k-blocks
F32 = mybir.dt.float32
F32R = mybir.dt.float32r
NEG = -1.0e30


def build_nc():
    nc = bacc.Bacc("TRN2", target_bir_lowering=False, debug=False, num_devices=NCORES)

    xT = nc.dram_tensor("xT", [C, N], F32R, kind="ExternalInput")
    wqkT = nc.dram_tensor("wqkT", [C, 2 * HPC * HD], F32R, kind="ExternalInput")
    bqk = nc.dram_tensor("bqk", [1, 2 * HPC * HD], F32R, kind="ExternalInput")
    wvT = nc.dram_tensor("wvT", [C, HPC * HD], F32R, kind="ExternalInput")
    bv = nc.dram_tensor("bv", [1, HPC * HD], F32R, kind="ExternalInput")
    wpT = nc.dram_tensor("wpT", [HPC * HD, C], F32R, kind="ExternalInput")
    masks = nc.dram_tensor("masks", [128, 4, QC], F32, kind="ExternalInput")
    onesd = nc.dram_tensor("onesd", [1, QC], F32R, kind="ExternalInput")
    vones = nc.dram_tensor("vones", [128, NKB * HPC], F32R, kind="ExternalInput")
    out = nc.dram_tensor("out", [C, N], F32, kind="ExternalOutput")

    rscratch = nc.dram_tensor("rscratch", [24, QC], F32)
    dscratch = nc.dram_tensor("dscratch", [24, QC], F32)
    debug = bool(os.environ.get("KDEBUG"))
    if debug:
        dbg_qk = nc.dram_tensor("dbg_qk", [128, 6 * N], F32, kind="ExternalOutput")
        dbg_v = nc.dram_tensor("dbg_v", [128, NKB * HPC * (HD + 1)], F32, kind="ExternalOutput")
        dbg_den = nc.dram_tensor("dbg_den", [24, QC], F32, kind="ExternalOutput")
        dbg_at = nc.dram_tensor("dbg_at", [128, 3 * N], F32, kind="ExternalOutput")
        dbg_pt = nc.dram_tensor("dbg_pt", [128, 8 * QC], F32, kind="ExternalOutput")
        dbg_st = nc.dram_tensor("dbg_st", [128, 8 * QC], F32, kind="ExternalOutput")

    CB = C // 128  # 6 contraction chunks

    with tile.TileContext(nc) as tc:
        with (
            tc.tile_pool(name="weights", bufs=1) as wpool,
            tc.tile_pool(name="big", bufs=1) as bigpool,
            tc.tile_pool(name="xc", bufs=2) as xcpool,
            tc.tile_pool(name="pt", bufs=3) as ptpool,
            tc.tile_pool(name="small", bufs=2) as smallpool,
        ):
            # ---- constant loads ----
            wqk_sb = wpool.tile([128, CB, 2 * HPC * HD], F32R)
            nc.sync.dma_start(wqk_sb, wqkT.ap().rearrange("(o p) m -> p o m", p=128))
            wv_sb = wpool.tile([128, CB, HPC * HD], F32R)
            nc.sync.dma_start(wv_sb, wvT.ap().rearrange("(o p) m -> p o m", p=128))
            wp_sb = wpool.tile([128, 3, C], F32R)
            nc.sync.dma_start(wp_sb, wpT.ap().rearrange("(o p) m -> p o m", p=128))
            bqk_sb = wpool.tile([1, 2 * HPC * HD], F32R)
            nc.sync.dma_start(bqk_sb, bqk.ap())
            bv_sb = wpool.tile([1, HPC * HD], F32R)
            nc.sync.dma_start(bv_sb, bv.ap())
            mask_sb = wpool.tile([128, 4, QC], F32)
            nc.sync.dma_start(mask_sb, masks.ap())
            ones_sb = wpool.tile([1, QC], F32R)
            nc.sync.dma_start(ones_sb, onesd.ap())

            # persistent activations
            qkT_sb = bigpool.tile([128, 2 * HPC // 2, N], F32R)  # [128, 6, 2048]
            v_sb = bigpool.tile([128, NKB, HPC, HD + 1], F32R)
            nc.sync.dma_start(
                v_sb[:, :, :, HD : HD + 1],
                vones.ap().rearrange("p (j h) -> p j h", j=NKB)[:, :, :, None],
            )
            # normalized attn^T reuses the (dead-after-use) Q^T blocks of qkT_sb
            attnT = qkT_sb
            den4 = bigpool.tile([128, 6 * QC], F32)
            den_sb = bigpool.tile([24, QC], F32)
            rec_sb = bigpool.tile([24, QC], F32)

            xT_r = xT.ap().rearrange("(o p) q -> p o q", p=128)

            # ---- phase 1: QKV projections ----
            ps1_ctx = tc.tile_pool(name="ps1", bufs=2, space="PSUM")
            ps_qk = ps1_ctx.__enter__()
            for qc in range(NQC):
                qs = slice(qc * QC, (qc + 1) * QC)
                xc = xcpool.tile([128, CB, QC], F32R, tag="xc")
                nc.sync.dma_start(xc, xT_r[:, :, qs])
                # Q^T / K^T: 6 M-chunks of 128 rows
                for m in range(2 * HPC * HD // 128):
                    ps = ps_qk.tile([128, QC], F32, tag="qk")
                    for c in range(CB):
                        nc.tensor.matmul(
                            ps,
                            lhsT=(wqk_sb[:, c, m * 128 : (m + 1) * 128]),
                            rhs=(xc[:, c, :]),
                            start=(c == 0),
                            stop=False,
                        )
                    nc.tensor.matmul(
                        ps,
                        lhsT=(bqk_sb[:, m * 128 : (m + 1) * 128]),
                        rhs=(ones_sb),
                        start=False,
                        stop=True,
                    )
                    nc.vector.tensor_copy(qkT_sb[:, m, qs], ps)
                # V: natural layout, k-blocks j = 4*qc + jj
                for jj in range(QC // 128):
                    j = qc * (QC // 128) + jj
                    psv = ps_qk.tile([128, HPC * HD], F32, tag="v")
                    for c in range(CB):
                        nc.tensor.matmul(
                            psv,
                            lhsT=(xc[:, c, jj * 128 : (jj + 1) * 128]),
                            rhs=(wv_sb[:, c, :]),
                            start=(c == 0),
                            stop=False,
                        )
                    nc.tensor.matmul(
                        psv,
                        lhsT=(ones_sb[:, 0:128]),
                        rhs=(bv_sb),
                        start=False,
                        stop=True,
                    )
                    nc.vector.tensor_copy(
                        v_sb[:, j, :, 0:HD],
                        psv.rearrange("p (h d) -> p h d", h=HPC),
                    )

            ps1_ctx.__exit__(None, None, None)

            if debug:
                nc.sync.dma_start(
                    dbg_qk.ap(), qkT_sb.rearrange("p a b -> p (a b)").bitcast(F32)
                )
                nc.sync.dma_start(
                    dbg_v.ap(), v_sb.rearrange("p a b c -> p (a b c)").bitcast(F32)
                )

            # ---- phase 2: attention per head pair ----
            ps_st_ctx = tc.tile_pool(name="ps_st", bufs=2, space="PSUM")
            ps_pv_ctx = tc.tile_pool(name="ps_pv", bufs=2, space="PSUM")
            ps_st = ps_st_ctx.__enter__()
            ps_pv = ps_pv_ctx.__enter__()
            for p in range(HPC // 2):
                for qc in range(NQC):
                    qs = slice(qc * QC, (qc + 1) * QC)
                    njb = 4 * qc + 4  # k-blocks for this q-chunk
                    nblk = 2 * njb  # 512-wide (j, head) blocks
                    psA = ps_pv.tile([HD + 1, QC], F32, tag="pv")
                    psB = ps_pv.tile([HD + 1, QC], F32, tag="pv")
                    st = pt = None
                    st_tiles = []
                    for t in range(nblk):
                        j, h2 = divmod(t, 2)
                        ti, slot = divmod(t, 3)
                        if slot == 0:
                            width = min(3, nblk - t) * QC
                            st = ps_st.tile([128, 3 * QC], F32, tag="st")
                            pt = ptpool.tile([128, 3 * QC], F32R, tag="pt")
                            st_tiles.append((st, pt, width, t))
                        ss = slice(slot * QC, (slot + 1) * QC)
                        pr = slice(64 * h2, 64 * (h2 + 1))
                        nc.tensor.matmul(
                            st[:, ss],
                            lhsT=(qkT_sb[pr, 3 + p, j * 128 : (j + 1) * 128]),
                            rhs=(qkT_sb[pr, p, qs]),
                            tile_position=(64 * h2, 0),
                            start=True,
                            stop=True,
                        )
                        if j >= 4 * qc:
                            nc.vector.tensor_add(
                                st[:, ss], st[:, ss], mask_sb[:, j - 4 * qc, :]
                            )
                        if debug and p == 0 and qc == 0:
                            dst_t = smallpool.tile([128, QC], F32, tag="dbgst")
                            nc.vector.tensor_copy(dst_t, st[:, ss])
                            nc.sync.dma_start(
                                dbg_st.ap()[:, t * QC : (t + 1) * QC], dst_t
                            )
                        if slot == 2 or t == nblk - 1:
                            w = slot * QC + QC
                            nc.scalar.activation(
                                pt[:, 0:w], st[:, 0:w],
                                mybir.ActivationFunctionType.Exp,
                                scale=SCALE,
                            )
                            base = t - slot
                            if debug and p == 0 and qc == 0:
                                nc.sync.dma_start(
                                    dbg_pt.ap()[:, base * QC : base * QC + w],
                                    pt[:, 0:w].bitcast(F32),
                                )
                            for t2 in range(base, t + 1):
                                j2, h22 = divmod(t2, 2)
                                s2 = slice((t2 - base) * QC, (t2 - base + 1) * QC)
                                nc.tensor.matmul(
                                    psA if h22 == 0 else psB,
                                    lhsT=(v_sb[:, j2, 2 * p + h22, :]),
                                    rhs=(pt[:, s2]),
                                    start=(j2 == 0),
                                    stop=(j2 == njb - 1),
                                )
                    # denominators out, unnormalized attn^T out
                    r = p * 8 + qc * 2
                    for h2, psX in ((0, psA), (1, psB)):
                        rr = r + h2
                        base, col = 32 * (rr // 6), rr % 6
                        nc.vector.tensor_copy(
                            den4[base : base + 1, col * QC : (col + 1) * QC],
                            psX[HD : HD + 1, :],
                        )
                    nc.vector.tensor_copy(attnT[0:64, p, qs], psA[0:HD, :])
                    nc.vector.tensor_copy(attnT[64:128, p, qs], psB[0:HD, :])

            ps_pv_ctx.__exit__(None, None, None)
            ps_st_ctx.__exit__(None, None, None)

            # ---- phase 2.5: normalize ----
            dsc_r = dscratch.ap().rearrange("(a c) q -> a c q", a=4)
            for a in range(4):
                nc.sync.dma_start(
                    dsc_r[a : a + 1, :, :],
                    den4[32 * a : 32 * a + 1, :].rearrange(
                        "p (c q) -> p c q", q=QC
                    ),
                )
            nc.sync.dma_start(den_sb, dscratch.ap())
            nc.vector.reciprocal(rec_sb, den_sb)
            nc.sync.dma_start(rscratch.ap(), rec_sb)
            for p in range(HPC // 2):
                for qc in range(NQC):
                    qs = slice(qc * QC, (qc + 1) * QC)
                    r = p * 8 + qc * 2
                    rb = smallpool.tile([128, QC], F32, tag="rb")
                    for h2 in range(2):
                        src = bass.AP(
                            tensor=rscratch,
                            offset=(r + h2) * QC,
                            ap=[[0, 64], [1, QC]],
                        )
                        nc.sync.dma_start(rb[64 * h2 : 64 * (h2 + 1), :], src)
                    nc.vector.tensor_mul(
                        attnT[:, p, qs], attnT[:, p, qs], rb
                    )

            if debug:
                nc.sync.dma_start(dbg_den.ap(), den_sb)
                nc.sync.dma_start(
                    dbg_at.ap(),
                    attnT[:, 0:3, :].rearrange("p a b -> p (a b)").bitcast(F32),
                )

            # ---- phase 3: output projection ----
            ps3_ctx = tc.tile_pool(name="ps3", bufs=2, space="PSUM")
            ps3 = ps3_ctx.__enter__()
            out_r = out.ap().rearrange("(o p) q -> p o q", p=128)
            for mc in range(C // 128):
                for qc in range(NQC):
                    qs = slice(qc * QC, (qc + 1) * QC)
                    pj = ps3.tile([128, QC], F32, tag="pj")
                    for c in range(3):
                        nc.tensor.matmul(
                            pj,
                            lhsT=(wp_sb[:, c, mc * 128 : (mc + 1) * 128]),
                            rhs=(attnT[:, c, qs]),
                            start=(c == 0),
                            stop=(c == 2),
                        )
                    oj = smallpool.tile([128, QC], F32, tag="oj")
                    nc.vector.tensor_copy(oj, pj)
                    nc.sync.dma_start(out_r[:, mc, qs], oj)
            ps3_ctx.__exit__(None, None, None)

    nc.compile()
    return nc


def make_in_maps(x, qkv_w, qkv_b):
    x = np.asarray(x, np.float32)
    qkv_w = np.asarray(qkv_w, np.float32)
    qkv_b = np.asarray(qkv_b, np.float32)

    # causal mask variants: v = j - 4*qc, cols q' in [0, 512)
    k_loc = np.arange(128)[:, None]
    qp = np.arange(QC)[None, :]
    mask = np.zeros((128, 4, QC), np.float32)
    for v in range(4):
        mask[:, v, :] = np.where(qp >= 128 * v + k_loc, 0.0, NEG)

    in_maps = []
    for c in range(NCORES):
        b = c // 2
        off = HPC * (c % 2)  # first global head
        rq = slice(off * HD, (off + HPC) * HD)  # rows within each of q/k/v blocks
        wq = qkv_w[rq, :]
        wk = qkv_w[C:2 * C, :][rq, :]
        wv = qkv_w[2 * C:3 * C, :][rq, :]
        in_maps.append({
            "xT": np.ascontiguousarray(x[b].T),
            "wqkT": np.ascontiguousarray(np.concatenate([wq, wk], 0).T),
            "bqk": np.concatenate(
                [qkv_b[:C][rq], qkv_b[C:2 * C][rq]]
            ).reshape(1, -1).copy(),
            "wvT": np.ascontiguousarray(wv.T),
            "bv": qkv_b[2 * C:3 * C][rq].reshape(1, -1).copy(),
            "masks": mask,
            "onesd": np.ones((1, QC), np.float32),
            "vones": np.ones((128, NKB * HPC), np.float32),
        })
    return in_maps


def assemble(results, proj_b, proj_w_unused=None):
    proj_b = np.asarray(proj_b, np.float32)
    full = np.empty((B, N, C), np.float32)
    for b in range(B):
        acc = results[2 * b]["out"] + results[2 * b + 1]["out"]
        full[b] = acc.T + proj_b
    return full


_NC = None
LAST_RES = None


def _get_nc():
    global _NC
    if _NC is None:
        _NC = build_nc()
    return _NC


def run(inputs, trace=False):
    nc = _get_nc()
    in_maps = make_in_maps(inputs["x"], inputs["qkv_w"], inputs["qkv_b"])
    # wpT needs proj_w
    proj_w = np.asarray(inputs["proj_w"], np.float32)
    for c in range(NCORES):
        off = HPC * (c % 2)
        in_maps[c]["wpT"] = np.ascontiguousarray(
            proj_w[:, off * HD : (off + HPC) * HD].T
        )
    res = run_bass_kernel_spmd(nc, in_maps, list(range(NCORES)), trace=trace)
    global LAST_RES
    LAST_RES = res
    out = assemble(res.results, inputs["proj_b"])
    return out, res.exec_time_ns


def kernel(x, qkv_w, qkv_b, proj_w, proj_b):
    out, _ = run(
        {"x": x, "qkv_w": qkv_w, "qkv_b": qkv_b, "proj_w": proj_w, "proj_b": proj_b},
        trace=bool(os.environ.get("KERNEL_TRACE")),
    )
    return out


# revision 21
# speedup vs baseline: 1.6927x; 1.6927x over previous
"""Causal multi-head attention block (B=4, N=2048, C=768, H=12) on 8 TRN2 cores.

Sharding: 48 (batch, head) pairs -> core c handles batch c//2 and heads
[0..5] (even c) or [6..11] (odd c). Each core runs an identical Bass program
on host-pre-sliced inputs; partial projection outputs are summed pairwise on
the host (plus proj bias).

Per-core kernel (bf16 matmul operands, fp32 PSUM accumulation; transposed
[feature, token] layout):
  - Q^T/K^T projection: lhsT = wqk^T chunks, rhs = x^T chunks; bias fused
    into the PSUM->SBUF copy via tensor_scalar_add (per-partition bias)
  - V projection in natural [token, feature] layout (lhsT = x^T chunk),
    bias via tensor_tensor add with a host-broadcast bias tile
  - S^T = K Q^T per head with 2-head tile_position row packing
  - exp on ScalarE (scale=1/sqrt(HD) folded into the activation affine)
  - causal masking post-exp: multiplicative {0,1} bf16 masks on P^T
  - PV: out^T = V_aug^T P^T with V augmented by a ones column so the softmax
    denominator accumulates in psum row 64
  - per-pair normalization: fast reciprocal + DMA partition-broadcast
  - output projection to out^T [768, 2048] per core
"""

import os

import numpy as np
import ml_dtypes

import concourse.bass as bass
import concourse.mybir as mybir
import concourse.tile as tile
from concourse import bacc
from concourse.bass_utils import run_bass_kernel_spmd

B, N, C = 4, 2048, 768
H, HD = 12, 64
HPC = 6  # heads per core
NP = HPC // 2  # head pairs per core
NCORES = 8
SCALE = HD ** -0.5
QC = 512  # q-chunk width
NQC = N // QC  # 4
NKB = N // 128  # 16 k-blocks
CB = C // 128  # 6 contraction chunks
F32 = mybir.dt.float32
BF16 = mybir.dt.bfloat16


def build_nc():
    nc = bacc.Bacc("TRN2", target_bir_lowering=False, debug=False, num_devices=NCORES)

    xT = nc.dram_tensor("xT", [C, N], BF16, kind="ExternalInput")
    wqkT = nc.dram_tensor("wqkT", [C, 2 * HPC * HD], BF16, kind="ExternalInput")
    wvT = nc.dram_tensor("wvT", [C, HPC * HD], BF16, kind="ExternalInput")
    wpT = nc.dram_tensor("wpT", [HPC * HD, C], BF16, kind="ExternalInput")
    bqk = nc.dram_tensor("bqk", [128, CB], F32, kind="ExternalInput")
    bvb = nc.dram_tensor("bvb", [128, HPC * HD], F32, kind="ExternalInput")
    mask01 = nc.dram_tensor("mask01", [128, 4, 2 * QC], BF16, kind="ExternalInput")
    vones = nc.dram_tensor("vones", [128, NKB * HPC], BF16, kind="ExternalInput")
    out = nc.dram_tensor("out", [C, N], F32, kind="ExternalOutput")

    rdram = [nc.dram_tensor(f"rdram{p}", [8, QC], F32) for p in range(NP)]

    with tile.TileContext(nc) as tc:
        with (
            tc.tile_pool(name="weights", bufs=1) as wpool,
            tc.tile_pool(name="big", bufs=1) as bigpool,
            tc.tile_pool(name="pt", bufs=3) as ptpool,
            tc.tile_pool(name="norm", bufs=2) as npool,
            tc.tile_pool(name="oj", bufs=2) as ojpool,
        ):
            # ---- constant loads ----
            wqk_sb = wpool.tile([128, CB, 2 * HPC * HD], BF16)
            nc.sync.dma_start(wqk_sb, wqkT.ap().rearrange("(o p) m -> p o m", p=128))
            wv_sb = wpool.tile([128, CB, HPC * HD], BF16)
            nc.sync.dma_start(wv_sb, wvT.ap().rearrange("(o p) m -> p o m", p=128))
            wp_sb = wpool.tile([128, 3, C], BF16)
            nc.sync.dma_start(wp_sb, wpT.ap().rearrange("(o p) m -> p o m", p=128))
            bqk_sb = wpool.tile([128, CB], F32)
            nc.sync.dma_start(bqk_sb, bqk.ap())
            bvb_sb = wpool.tile([128, HPC * HD], F32)
            nc.sync.dma_start(bvb_sb, bvb.ap())
            mask_sb = wpool.tile([128, 4, 2 * QC], BF16)
            nc.sync.dma_start(mask_sb, mask01.ap())

            # persistent activations
            xT_sb = bigpool.tile([128, CB, N], BF16)
            xT_r = xT.ap().rearrange("(o p) q -> p o q", p=128)
            for qc in range(NQC):
                qs = slice(qc * QC, (qc + 1) * QC)
                nc.sync.dma_start(xT_sb[:, :, qs], xT_r[:, :, qs])
            qkT_sb = bigpool.tile([128, 2 * NP, N], BF16)  # [128, 6, 2048]
            v_sb = bigpool.tile([128, NKB, HPC, HD + 1], BF16)
            nc.sync.dma_start(
                v_sb[:, :, :, HD : HD + 1],
                vones.ap().rearrange("p (j h) -> p j h", j=NKB)[:, :, :, None],
            )
            # normalized attn^T reuses the (dead-after-use) Q^T blocks of qkT_sb
            attnT = qkT_sb

            # ---- phase 1: QKV projections ----
            ps1_ctx = tc.tile_pool(name="ps1", bufs=2, space="PSUM")
            ps_qk = ps1_ctx.__enter__()
            for qc in range(NQC):
                qs = slice(qc * QC, (qc + 1) * QC)
                # Q^T / K^T: 6 M-chunks of 128 rows
                for m in range(2 * HPC * HD // 128):
                    ps = ps_qk.tile([128, QC], F32, tag="qk")
                    for c in range(CB):
                        nc.tensor.matmul(
                            ps,
                            lhsT=wqk_sb[:, c, m * 128 : (m + 1) * 128],
                            rhs=xT_sb[:, c, qs],
                            start=(c == 0),
                            stop=(c == CB - 1),
                        )
                    nc.vector.tensor_scalar_add(
                        qkT_sb[:, m, qs], ps, bqk_sb[:, m : m + 1]
                    )
                # V: natural layout, k-blocks j = 4*qc + jj
                for jj in range(QC // 128):
                    j = qc * (QC // 128) + jj
                    psv = ps_qk.tile([128, HPC * HD], F32, tag="v")
                    for c in range(CB):
                        nc.tensor.matmul(
                            psv,
                            lhsT=xT_sb[:, c, j * 128 : (j + 1) * 128],
                            rhs=wv_sb[:, c, :],
                            start=(c == 0),
                            stop=(c == CB - 1),
                        )
                    nc.vector.tensor_add(
                        v_sb[:, j, :, 0:HD],
                        psv.rearrange("p (h d) -> p h d", h=HPC),
                        bvb_sb.rearrange("p (h d) -> p h d", h=HPC),
                    )
            ps1_ctx.__exit__(None, None, None)

            # ---- phase 2: attention per head pair ----
            ps_st_ctx = tc.tile_pool(name="ps_st", bufs=2, space="PSUM")
            ps_pv_ctx = tc.tile_pool(name="ps_pv", bufs=2, space="PSUM")
            ps_st = ps_st_ctx.__enter__()
            ps_pv = ps_pv_ctx.__enter__()
            for p in range(NP):
                # per-pair denominator staging: qc -> partition base 32*qc,
                # head -> column half
                dstg = npool.tile([128, 2 * QC], F32, tag="dstg")
                rstg = npool.tile([128, 2 * QC], F32, tag="rstg")
                for qc in range(NQC):
                    qs = slice(qc * QC, (qc + 1) * QC)
                    njb = 4 * qc + 4  # k-blocks for this q-chunk
                    nblk = 2 * njb  # 512-wide (j, head) blocks
                    psA = ps_pv.tile([HD + 1, QC], F32, tag="pv")
                    psB = ps_pv.tile([HD + 1, QC], F32, tag="pv")
                    st = pt = None
                    for t in range(nblk):
                        j, h2 = divmod(t, 2)
                        ti, slot = divmod(t, 3)
                        if slot == 0:
                            st = ps_st.tile([128, 3 * QC], F32, tag="st")
                            pt = ptpool.tile([128, 3 * QC], BF16, tag="pt")
                        ss = slice(slot * QC, (slot + 1) * QC)
                        pr = slice(64 * h2, 64 * (h2 + 1))
                        nc.tensor.matmul(
                            st[:, ss],
                            lhsT=qkT_sb[pr, NP + p, j * 128 : (j + 1) * 128],
                            rhs=qkT_sb[pr, p, qs],
                            tile_position=(64 * h2, 0),
                            start=True,
                            stop=True,
                        )
                        if slot == 2 or t == nblk - 1:
                            w = slot * QC + QC
                            base = t - slot
                            nc.scalar.activation(
                                pt[:, 0:w], st[:, 0:w],
                                mybir.ActivationFunctionType.Exp,
                                scale=SCALE,
                            )
                            # post-exp multiplicative causal masks on the
                            # diagonal blocks of this tile
                            t2 = base
                            while t2 <= t:
                                j2, h22 = divmod(t2, 2)
                                s0 = (t2 - base) * QC
                                if j2 >= 4 * qc:
                                    v = j2 - 4 * qc
                                    if h22 == 0 and t2 + 1 <= t:
                                        nc.vector.tensor_mul(
                                            pt[:, s0 : s0 + 2 * QC],
                                            pt[:, s0 : s0 + 2 * QC],
                                            mask_sb[:, v, :],
                                        )
                                        t2 += 2
                                        continue
                                    nc.vector.tensor_mul(
                                        pt[:, s0 : s0 + QC],
                                        pt[:, s0 : s0 + QC],
                                        mask_sb[:, v, 0:QC],
                                    )
                                t2 += 1
                            # PV accumulation for the blocks of this tile
                            for t2 in range(base, t + 1):
                                j2, h22 = divmod(t2, 2)
                                s2 = slice((t2 - base) * QC, (t2 - base + 1) * QC)
                                nc.tensor.matmul(
                                    psA if h22 == 0 else psB,
                                    lhsT=v_sb[:, j2, 2 * p + h22, :],
                                    rhs=pt[:, s2],
                                    start=(j2 == 0),
                                    stop=(j2 == njb - 1),
                                )
                    # stash denominators; copy unnormalized attn^T out
                    bp = 32 * qc
                    nc.vector.tensor_copy(
                        dstg[bp : bp + 1, 0:QC], psA[HD : HD + 1, :]
                    )
                    nc.vector.tensor_copy(
                        dstg[bp : bp + 1, QC : 2 * QC], psB[HD : HD + 1, :]
                    )
                    nc.vector.tensor_copy(attnT[0:64, p, qs], psA[0:HD, :])
                    nc.vector.tensor_copy(attnT[64:128, p, qs], psB[0:HD, :])
                # ---- per-pair normalization ----
                nc.vector.reciprocal_approx_fast(rstg, dstg)
                rd = rdram[p].ap().rearrange("(a b) q -> a b q", b=2)
                for qc in range(NQC):
                    nc.sync.dma_start(
                        rd[qc : qc + 1],
                        rstg[32 * qc : 32 * qc + 1, :].rearrange(
                            "p (b q) -> p b q", b=2
                        ),
                    )
                for qc in range(NQC):
                    qs = slice(qc * QC, (qc + 1) * QC)
                    rb = npool.tile([128, QC], F32, tag="rb")
                    for h2 in range(2):
                        src = bass.AP(
                            tensor=rdram[p],
                            offset=(2 * qc + h2) * QC,
                            ap=[[0, 64], [1, QC]],
                        )
                        nc.sync.dma_start(rb[64 * h2 : 64 * (h2 + 1), :], src)
                    nc.vector.tensor_mul(attnT[:, p, qs], attnT[:, p, qs], rb)
            ps_pv_ctx.__exit__(None, None, None)
            ps_st_ctx.__exit__(None, None, None)

            # ---- phase 3: output projection ----
            ps3_ctx = tc.tile_pool(name="ps3", bufs=2, space="PSUM")
            ps3 = ps3_ctx.__enter__()
            out_r = out.ap().rearrange("(o p) q -> p o q", p=128)
            for mc in range(C // 128):
                for qc in range(NQC):
                    qs = slice(qc * QC, (qc + 1) * QC)
                    pj = ps3.tile([128, QC], F32, tag="pj")
                    for c in range(3):
                        nc.tensor.matmul(
                            pj,
                            lhsT=wp_sb[:, c, mc * 128 : (mc + 1) * 128],
                            rhs=attnT[:, c, qs],
                            start=(c == 0),
                            stop=(c == 2),
                        )
                    oj = ojpool.tile([128, QC], F32, tag="oj")
                    nc.vector.tensor_copy(oj, pj)
                    nc.sync.dma_start(out_r[:, mc, qs], oj)
            ps3_ctx.__exit__(None, None, None)

    nc.compile()
    return nc


def make_in_maps(x, qkv_w, qkv_b, proj_w):
    bf = ml_dtypes.bfloat16
    x = np.asarray(x, np.float32)
    qkv_w = np.asarray(qkv_w, np.float32)
    qkv_b = np.asarray(qkv_b, np.float32)
    proj_w = np.asarray(proj_w, np.float32)

    # multiplicative causal mask variants: v = j - 4*qc, cols q' in [0, 512),
    # duplicated across both 512 halves (per-head blocks)
    k_loc = np.arange(128)[:, None]
    qp = np.arange(QC)[None, :]
    m1 = np.zeros((128, 4, QC), np.float32)
    for v in range(4):
        m1[:, v, :] = (qp >= 128 * v + k_loc).astype(np.float32)
    mask = np.concatenate([m1, m1], axis=2).astype(bf)

    in_maps = []
    for c in range(NCORES):
        b = c // 2
        off = HPC * (c % 2)  # first global head
        rq = slice(off * HD, (off + HPC) * HD)
        wq = qkv_w[rq, :]
        wk = qkv_w[C:2 * C, :][rq, :]
        wv = qkv_w[2 * C:3 * C, :][rq, :]
        bqk_cols = np.concatenate([qkv_b[:C][rq], qkv_b[C:2 * C][rq]])
        in_maps.append({
            "xT": np.ascontiguousarray(x[b].T).astype(bf),
            "wqkT": np.ascontiguousarray(np.concatenate([wq, wk], 0).T).astype(bf),
            "wvT": np.ascontiguousarray(wv.T).astype(bf),
            "wpT": np.ascontiguousarray(
                proj_w[:, off * HD : (off + HPC) * HD].T
            ).astype(bf),
            "bqk": np.ascontiguousarray(bqk_cols.reshape(CB, 128).T).copy(),
            "bvb": np.broadcast_to(
                qkv_b[2 * C:3 * C][rq][None, :], (128, HPC * HD)
            ).copy(),
            "mask01": mask,
            "vones": np.ones((128, NKB * HPC), bf),
        })
    return in_maps


def assemble(results, proj_b):
    proj_b = np.asarray(proj_b, np.float32)
    full = np.empty((B, N, C), np.float32)
    for b in range(B):
        acc = results[2 * b]["out"] + results[2 * b + 1]["out"]
        full[b] = acc.T + proj_b
    return full


_NC = None
LAST_RES = None


def _get_nc():
    global _NC
    if _NC is None:
        _NC = build_nc()
    return _NC


def run(inputs, trace=False):
    nc = _get_nc()
    in_maps = make_in_maps(
        inputs["x"], inputs["qkv_w"], inputs["qkv_b"], inputs["proj_w"]
    )
    res = run_bass_kernel_spmd(nc, in_maps, list(range(NCORES)), trace=trace)
    global LAST_RES
    LAST_RES = res
    out = assemble(res.results, inputs["proj_b"])
    return out, res.exec_time_ns


def kernel(x, qkv_w, qkv_b, proj_w, proj_b):
    out, _ = run(
        {"x": x, "qkv_w": qkv_w, "qkv_b": qkv_b, "proj_w": proj_w, "proj_b": proj_b},
        trace=bool(os.environ.get("KERNEL_TRACE")),
    )
    return out


# revision 24
# speedup vs baseline: 1.7181x; 1.0150x over previous
"""Causal multi-head attention block (B=4, N=2048, C=768, H=12) on 8 TRN2 cores.

Sharding: 48 (batch, head) pairs -> core c handles batch c//2 and heads
[0..5] (even c) or [6..11] (odd c). Each core runs an identical Bass program
on host-pre-sliced inputs; partial projection outputs are summed pairwise on
the host (plus proj bias).

Per-core kernel (bf16 matmul operands, fp32 PSUM accumulation; transposed
[feature, token] layout):
  - Q^T/K^T projection: lhsT = wqk^T chunks, rhs = x^T chunks; bias fused
    into the PSUM->SBUF copy via tensor_scalar_add (per-partition bias)
  - V projection in natural [token, feature] layout (lhsT = x^T chunk),
    bias via tensor_tensor add with a host-broadcast bias tile
  - S^T = K Q^T per head with 2-head tile_position row packing
  - exp on ScalarE (scale=1/sqrt(HD) folded into the activation affine)
  - causal masking post-exp: multiplicative {0,1} bf16 masks on P^T
  - PV: out^T = V_aug^T P^T with V augmented by a ones column so the softmax
    denominator accumulates in psum row 64
  - per-pair normalization: fast reciprocal + DMA partition-broadcast
  - output projection to out^T [768, 2048] per core
"""

import os

import numpy as np
import ml_dtypes

import concourse.bass as bass
import concourse.mybir as mybir
import concourse.tile as tile
from concourse import bacc
from concourse.bass_utils import run_bass_kernel_spmd

B, N, C = 4, 2048, 768
H, HD = 12, 64
HPC = 6  # heads per core
NP = HPC // 2  # head pairs per core
NCORES = 8
SCALE = HD ** -0.5
QC = 512  # q-chunk width
NQC = N // QC  # 4
NKB = N // 128  # 16 k-blocks
CB = C // 128  # 6 contraction chunks
F32 = mybir.dt.float32
BF16 = mybir.dt.bfloat16


def build_nc():
    nc = bacc.Bacc("TRN2", target_bir_lowering=False, debug=False, num_devices=NCORES)

    xT = nc.dram_tensor("xT", [C, N], BF16, kind="ExternalInput")
    wqkT = nc.dram_tensor("wqkT", [C, 2 * HPC * HD], BF16, kind="ExternalInput")
    wvT = nc.dram_tensor("wvT", [C, HPC * HD], BF16, kind="ExternalInput")
    wpT = nc.dram_tensor("wpT", [HPC * HD, C], BF16, kind="ExternalInput")
    bqk = nc.dram_tensor("bqk", [128, CB], F32, kind="ExternalInput")
    bvb = nc.dram_tensor("bvb", [128, HPC * HD], F32, kind="ExternalInput")
    mask01 = nc.dram_tensor("mask01", [128, 4, 2 * QC], BF16, kind="ExternalInput")
    vones = nc.dram_tensor("vones", [128, NKB * HPC], BF16, kind="ExternalInput")
    out = nc.dram_tensor("out", [C, N], F32, kind="ExternalOutput")


    with tile.TileContext(nc) as tc:
        with (
            tc.tile_pool(name="weights", bufs=1) as wpool,
            tc.tile_pool(name="big", bufs=1) as bigpool,
            tc.tile_pool(name="pt", bufs=3) as ptpool,
            tc.tile_pool(name="norm", bufs=2) as npool,
            tc.tile_pool(name="oj", bufs=2) as ojpool,
            tc.tile_pool(name="dram", bufs=2, space="DRAM") as drampool,
        ):
            # ---- critical-path loads first: wqk chunks + x^T slices ----
            wqk_sb = wpool.tile([128, CB, 2 * HPC * HD], BF16)
            wqk_r = wqkT.ap().rearrange("(o p) m -> p o m", p=128)
            xT_sb = bigpool.tile([128, CB, N], BF16)
            xT_r = xT.ap().rearrange("(o p) q -> p o q", p=128)
            nc.sync.dma_start(xT_sb[:, :, 0:QC], xT_r[:, :, 0:QC])
            for c in range(CB):
                nc.sync.dma_start(wqk_sb[:, c, :], wqk_r[:, c, :])
            bqk_sb = wpool.tile([128, CB], F32)
            nc.sync.dma_start(bqk_sb, bqk.ap())
            wv_sb = wpool.tile([128, CB, HPC * HD], BF16)
            nc.sync.dma_start(wv_sb, wvT.ap().rearrange("(o p) m -> p o m", p=128))
            bvb_sb = wpool.tile([128, HPC * HD], F32)
            nc.sync.dma_start(bvb_sb, bvb.ap())
            for qc in range(1, NQC):
                qs = slice(qc * QC, (qc + 1) * QC)
                nc.sync.dma_start(xT_sb[:, :, qs], xT_r[:, :, qs])
            wp_sb = wpool.tile([128, 3, C], BF16)
            nc.sync.dma_start(wp_sb, wpT.ap().rearrange("(o p) m -> p o m", p=128))
            mask_sb = wpool.tile([128, 4, 2 * QC], BF16)
            nc.sync.dma_start(mask_sb, mask01.ap())
            qkT_sb = bigpool.tile([128, 2 * NP, N], BF16)  # [128, 6, 2048]
            v_sb = bigpool.tile([128, NKB, HPC, HD + 1], BF16)
            nc.sync.dma_start(
                v_sb[:, :, :, HD : HD + 1],
                vones.ap().rearrange("p (j h) -> p j h", j=NKB)[:, :, :, None],
            )
            # normalized attn^T reuses the (dead-after-use) Q^T blocks of qkT_sb
            attnT = qkT_sb

            # ---- phase 1: QKV projections ----
            ps1_ctx = tc.tile_pool(name="ps1", bufs=4, space="PSUM")
            ps_qk = ps1_ctx.__enter__()
            for qc in range(NQC):
                qs = slice(qc * QC, (qc + 1) * QC)
                # Q^T / K^T: 6 M-chunks of 128 rows
                for m in range(2 * HPC * HD // 128):
                    ps = ps_qk.tile([128, QC], F32, tag="qk")
                    for c in range(CB):
                        nc.tensor.matmul(
                            ps,
                            lhsT=wqk_sb[:, c, m * 128 : (m + 1) * 128],
                            rhs=xT_sb[:, c, qs],
                            start=(c == 0),
                            stop=(c == CB - 1),
                        )
                    nc.vector.tensor_scalar_add(
                        qkT_sb[:, m, qs], ps, bqk_sb[:, m : m + 1]
                    )
                # V: natural layout, k-blocks j = 4*qc + jj
                for jj in range(QC // 128):
                    j = qc * (QC // 128) + jj
                    psv = ps_qk.tile([128, HPC * HD], F32, tag="v")
                    for c in range(CB):
                        nc.tensor.matmul(
                            psv,
                            lhsT=xT_sb[:, c, j * 128 : (j + 1) * 128],
                            rhs=wv_sb[:, c, :],
                            start=(c == 0),
                            stop=(c == CB - 1),
                        )
                    nc.vector.tensor_add(
                        v_sb[:, j, :, 0:HD],
                        psv.rearrange("p (h d) -> p h d", h=HPC),
                        bvb_sb.rearrange("p (h d) -> p h d", h=HPC),
                    )
            ps1_ctx.__exit__(None, None, None)

            # ---- phase 2: attention per head pair ----
            ps_st_ctx = tc.tile_pool(name="ps_st", bufs=2, space="PSUM")
            ps_pv_ctx = tc.tile_pool(name="ps_pv", bufs=2, space="PSUM")
            ps_st = ps_st_ctx.__enter__()
            ps_pv = ps_pv_ctx.__enter__()
            for p in range(NP):
                # per-pair denominator staging: qc -> partition base 32*qc,
                # head -> column half
                dstg = npool.tile([128, 2 * QC], F32, tag="dstg")
                rstg = npool.tile([128, 2 * QC], F32, tag="rstg")
                for qc in range(NQC):
                    qs = slice(qc * QC, (qc + 1) * QC)
                    njb = 4 * qc + 4  # k-blocks for this q-chunk
                    nblk = 2 * njb  # 512-wide (j, head) blocks
                    psA = ps_pv.tile([HD + 1, QC], F32, tag="pv")
                    psB = ps_pv.tile([HD + 1, QC], F32, tag="pv")
                    st = pt = None
                    for t in range(nblk):
                        j, h2 = divmod(t, 2)
                        ti, slot = divmod(t, 3)
                        if slot == 0:
                            st = ps_st.tile([128, 3 * QC], F32, tag="st")
                            pt = ptpool.tile([128, 3 * QC], BF16, tag="pt")
                        ss = slice(slot * QC, (slot + 1) * QC)
                        pr = slice(64 * h2, 64 * (h2 + 1))
                        nc.tensor.matmul(
                            st[:, ss],
                            lhsT=qkT_sb[pr, NP + p, j * 128 : (j + 1) * 128],
                            rhs=qkT_sb[pr, p, qs],
                            tile_position=(64 * h2, 0),
                            start=True,
                            stop=True,
                        )
                        if slot == 2 or t == nblk - 1:
                            w = slot * QC + QC
                            base = t - slot
                            nc.scalar.activation(
                                pt[:, 0:w], st[:, 0:w],
                                mybir.ActivationFunctionType.Exp,
                                scale=SCALE,
                            )
                            # post-exp multiplicative causal masks on the
                            # diagonal blocks of this tile
                            t2 = base
                            while t2 <= t:
                                j2, h22 = divmod(t2, 2)
                                s0 = (t2 - base) * QC
                                if j2 >= 4 * qc:
                                    v = j2 - 4 * qc
                                    if h22 == 0 and t2 + 1 <= t:
                                        nc.vector.tensor_mul(
                                            pt[:, s0 : s0 + 2 * QC],
                                            pt[:, s0 : s0 + 2 * QC],
                                            mask_sb[:, v, :],
                                        )
                                        t2 += 2
                                        continue
                                    nc.vector.tensor_mul(
                                        pt[:, s0 : s0 + QC],
                                        pt[:, s0 : s0 + QC],
                                        mask_sb[:, v, 0:QC],
                                    )
                                t2 += 1
                            # PV accumulation for the blocks of this tile
                            for t2 in range(base, t + 1):
                                j2, h22 = divmod(t2, 2)
                                s2 = slice((t2 - base) * QC, (t2 - base + 1) * QC)
                                nc.tensor.matmul(
                                    psA if h22 == 0 else psB,
                                    lhsT=v_sb[:, j2, 2 * p + h22, :],
                                    rhs=pt[:, s2],
                                    start=(j2 == 0),
                                    stop=(j2 == njb - 1),
                                )
                    # stash denominators; copy unnormalized attn^T out
                    bp = 32 * qc
                    nc.vector.tensor_copy(
                        dstg[bp : bp + 1, 0:QC], psA[HD : HD + 1, :]
                    )
                    nc.vector.tensor_copy(
                        dstg[bp : bp + 1, QC : 2 * QC], psB[HD : HD + 1, :]
                    )
                    nc.vector.tensor_copy(attnT[0:64, p, qs], psA[0:HD, :])
                    nc.vector.tensor_copy(attnT[64:128, p, qs], psB[0:HD, :])
                # ---- per-pair normalization ----
                nc.vector.reciprocal_approx_fast(rstg, dstg)
                rdram = drampool.tile([8, QC], F32, tag="rdram")
                rd = rdram.rearrange("(a b) q -> a b q", b=2)
                for qc in range(NQC):
                    nc.sync.dma_start(
                        rd[qc : qc + 1],
                        rstg[32 * qc : 32 * qc + 1, :].rearrange(
                            "p (b q) -> p b q", b=2
                        ),
                    )
                for qc in range(NQC):
                    qs = slice(qc * QC, (qc + 1) * QC)
                    rb = npool.tile([128, QC], F32, tag="rb")
                    for h2 in range(2):
                        src = bass.AP(
                            tensor=rdram.tensor,
                            offset=rdram.offset + (2 * qc + h2) * QC,
                            ap=[[0, 64], [1, QC]],
                        )
                        nc.sync.dma_start(rb[64 * h2 : 64 * (h2 + 1), :], src)
                    nc.vector.tensor_mul(attnT[:, p, qs], attnT[:, p, qs], rb)
            ps_pv_ctx.__exit__(None, None, None)
            ps_st_ctx.__exit__(None, None, None)

            # ---- phase 3: output projection ----
            ps3_ctx = tc.tile_pool(name="ps3", bufs=2, space="PSUM")
            ps3 = ps3_ctx.__enter__()
            out_r = out.ap().rearrange("(o p) q -> p o q", p=128)
            for mc in range(C // 128):
                for qc in range(NQC):
                    qs = slice(qc * QC, (qc + 1) * QC)
                    pj = ps3.tile([128, QC], F32, tag="pj")
                    for c in range(3):
                        nc.tensor.matmul(
                            pj,
                            lhsT=wp_sb[:, c, mc * 128 : (mc + 1) * 128],
                            rhs=attnT[:, c, qs],
                            start=(c == 0),
                            stop=(c == 2),
                        )
                    oj = ojpool.tile([128, QC], F32, tag="oj")
                    nc.vector.tensor_copy(oj, pj)
                    nc.sync.dma_start(out_r[:, mc, qs], oj)
            ps3_ctx.__exit__(None, None, None)

    nc.compile()
    return nc


def make_in_maps(x, qkv_w, qkv_b, proj_w):
    bf = ml_dtypes.bfloat16
    x = np.asarray(x, np.float32)
    qkv_w = np.asarray(qkv_w, np.float32)
    qkv_b = np.asarray(qkv_b, np.float32)
    proj_w = np.asarray(proj_w, np.float32)

    # multiplicative causal mask variants: v = j - 4*qc, cols q' in [0, 512),
    # duplicated across both 512 halves (per-head blocks)
    k_loc = np.arange(128)[:, None]
    qp = np.arange(QC)[None, :]
    m1 = np.zeros((128, 4, QC), np.float32)
    for v in range(4):
        m1[:, v, :] = (qp >= 128 * v + k_loc).astype(np.float32)
    mask = np.concatenate([m1, m1], axis=2).astype(bf)

    in_maps = []
    for c in range(NCORES):
        b = c // 2
        off = HPC * (c % 2)  # first global head
        rq = slice(off * HD, (off + HPC) * HD)
        wq = qkv_w[rq, :]
        wk = qkv_w[C:2 * C, :][rq, :]
        wv = qkv_w[2 * C:3 * C, :][rq, :]
        bqk_cols = np.concatenate([qkv_b[:C][rq], qkv_b[C:2 * C][rq]])
        in_maps.append({
            "xT": np.ascontiguousarray(x[b].T).astype(bf),
            "wqkT": np.ascontiguousarray(np.concatenate([wq, wk], 0).T).astype(bf),
            "wvT": np.ascontiguousarray(wv.T).astype(bf),
            "wpT": np.ascontiguousarray(
                proj_w[:, off * HD : (off + HPC) * HD].T
            ).astype(bf),
            "bqk": np.ascontiguousarray(bqk_cols.reshape(CB, 128).T).copy(),
            "bvb": np.broadcast_to(
                qkv_b[2 * C:3 * C][rq][None, :], (128, HPC * HD)
            ).copy(),
            "mask01": mask,
            "vones": np.ones((128, NKB * HPC), bf),
        })
    return in_maps


def assemble(results, proj_b):
    proj_b = np.asarray(proj_b, np.float32)
    full = np.empty((B, N, C), np.float32)
    for b in range(B):
        acc = results[2 * b]["out"] + results[2 * b + 1]["out"]
        full[b] = acc.T + proj_b
    return full


_NC = None
LAST_RES = None


def _get_nc():
    global _NC
    if _NC is None:
        _NC = build_nc()
    return _NC


def run(inputs, trace=False):
    nc = _get_nc()
    in_maps = make_in_maps(
        inputs["x"], inputs["qkv_w"], inputs["qkv_b"], inputs["proj_w"]
    )
    res = run_bass_kernel_spmd(nc, in_maps, list(range(NCORES)), trace=trace)
    global LAST_RES
    LAST_RES = res
    out = assemble(res.results, inputs["proj_b"])
    return out, res.exec_time_ns


def kernel(x, qkv_w, qkv_b, proj_w, proj_b):
    out, _ = run(
        {"x": x, "qkv_w": qkv_w, "qkv_b": qkv_b, "proj_w": proj_w, "proj_b": proj_b},
        trace=bool(os.environ.get("KERNEL_TRACE")),
    )
    return out
